# revision 28
# baseline (speedup 1.0000x reference)
"""Grouped-Query Attention (B=2, T=2048, C=2048, 16 Q heads / 4 KV heads,
D=128) on 8 Trainium2 NeuronCores.

Sharding: core (b, g) for b in {0,1}, g in {0..3} handles batch b and KV head
g (= query heads 4g..4g+3). Each core computes its 4 heads' attention plus the
partial output projection against its 512-row slice of Wo; the host sums the
4 partials per batch (the "all-reduce" of the o_proj, done in numpy).

All matmul operands are bf16 (host-cast); PSUM accumulation stays fp32, so
the only precision loss is input rounding (~4e-3 rel err vs the 2e-2 gate).

Layout/scheduling notes (from trace analysis):
- One PSUM pool with 8 [128,512]-f32 bank tags reused across stages (no
  mid-kernel pool releases -> no cross-stage drain bubbles; the PE pstate
  ramp resets on idle gaps, so a dense PE queue is worth ~1.5x clock).
- Startup DMAs interleaved per contraction chunk (wq/wk/wv/x) so the first
  projection matmul unblocks after ~4 transfers instead of all weights.
- Softmax denominator is computed REPLICATED across all 128 partitions
  (lhsT = all-ones [128,128]) so the reciprocal runs as a full-width DVE op
  (~0.65us) instead of a 1-partition op (3.3us) + GpSimd partition
  broadcast; the per-head tail stall on the PE disappears.
- Stage-2 software pipeline: score matmuls run 2 s-tiles ahead of the
  dependent den/PV matmuls so the PE never waits on ScalarE's exp.
- Diagonal (causal-boundary) s-tiles only compute the t-window right of the
  diagonal plus one shared [128,128] triangular 0/1 multiply.
"""
import sys

sys.path.insert(0, "/opt/trn_rl_repo")

import numpy as np
import ml_dtypes

B, T, C = 2, 2048, 2048
NUM_HEADS, NUM_KV_HEADS, HEAD_DIM = 16, 4, 128
G = NUM_HEADS // NUM_KV_HEADS  # 4 query heads per core
SCALE = float(HEAD_DIM) ** -0.5
TB = 512  # t-block (matmul moving free dim)
NTB = T // TB  # 4
ST = 128  # s-tile
NST = T // ST  # 16
NCT = C // 128  # 16 contraction tiles
LA = 3  # stage-2 score-matmul lookahead (s-tiles in flight past exp)

SWAP_MASK = [i ^ 1 for i in range(32)]
BF = ml_dtypes.bfloat16

_nc_cache: dict = {}

# plan entry kinds
FULL, DIAG, GEN = 0, 1, 2


def _classify_mask(mask2d: np.ndarray):
    """mask2d[t, s] bool. Returns (plan, mask_tiles).

    plan[tb] = tuple of (s_tile_idx, w0, kind, mask_id). w0 is the t-window
    start within the t-block (columns < w0 are entirely masked for this
    s-tile). kind: FULL (no mask work), DIAG (shared lower-triangular 0/1
    multiply on the first 128 window columns), GEN (per-tile 0/1 multiply
    over the whole window; mask_id indexes mask_tiles)."""
    tri = (np.arange(ST)[:, None] <= np.arange(ST)[None, :])
    plan = []
    uniq: dict = {}
    tiles = []
    for tb in range(NTB):
        sub_t = mask2d[tb * TB : (tb + 1) * TB]  # [TB(t), T(s)]
        entries = []
        for s in range(NST):
            sub = sub_t[:, s * ST : (s + 1) * ST]  # [TB(t), ST(s)]
            if sub.all():
                entries.append((s, 0, FULL, None))
                continue
            if not sub.any():
                continue
            m = sub.T  # [s, t]
            w0 = 0
            while w0 + ST <= TB and not m[:, w0 : w0 + ST].any():
                w0 += ST
            win = m[:, w0:]
            if (
                win.shape[1] >= ST
                and (win[:, :ST] == tri).all()
                and win[:, ST:].all()
            ):
                entries.append((s, w0, DIAG, None))
                continue
            tile_m = np.zeros((ST, TB), dtype=np.float32)
            tile_m[:, : TB - w0] = win.astype(np.float32)
            key = (w0, tile_m.tobytes())
            mid = uniq.get(key)
            if mid is None:
                mid = len(tiles)
                uniq[key] = mid
                tiles.append(tile_m)
            entries.append((s, w0, GEN, mid))
        plan.append(tuple(entries))
    mask_tiles = (
        np.stack(tiles) if tiles else np.zeros((0, ST, TB), dtype=np.float32)
    )
    return tuple(plan), mask_tiles


def _build(plan, n_masks):
    import concourse.bacc as bacc
    import concourse.mybir as mybir
    import concourse.tile as tile

    F32 = mybir.dt.float32
    BF16 = mybir.dt.bfloat16
    Exp = mybir.ActivationFunctionType.Exp

    nc = bacc.Bacc()

    xT_d = nc.declare_dram_parameter("xT", [C, T], BF16, isOutput=False)
    # wqkv = [Wq | Wk | Wv] columns, one DMA per 128-row chunk
    wqkv_d = nc.declare_dram_parameter(
        "wqkv", [C, (G + 2) * HEAD_DIM], BF16, isOutput=False
    )
    wo_d = nc.declare_dram_parameter("wo", [G * HEAD_DIM, C], BF16, isOutput=False)
    on_d = nc.declare_dram_parameter("ones", [128, 128], BF16, isOutput=False)
    id_d = nc.declare_dram_parameter("ident", [128, 128], BF16, isOutput=False)
    tr_d = nc.declare_dram_parameter("tri", [ST, ST], BF16, isOutput=False)
    ct_d = nc.declare_dram_parameter("ctab", [HEAD_DIM, T], BF16, isOutput=False)
    st_d = nc.declare_dram_parameter("stab", [HEAD_DIM, T], BF16, isOutput=False)
    if n_masks:
        mk_d = nc.declare_dram_parameter(
            "masks", [n_masks * ST, TB], BF16, isOutput=False
        )
    out_d = nc.declare_dram_parameter("out", [T, C], BF16, isOutput=True)

    with tile.TileContext(nc) as tc:
        const = tc.alloc_tile_pool(name="const", bufs=1)
        wop = tc.alloc_tile_pool(name="wop", bufs=1)
        qkv = tc.alloc_tile_pool(name="qkv", bufs=1)
        xp = tc.alloc_tile_pool(name="xp", bufs=1)

        # --- interleaved startup DMAs: per-chunk weights + x so the first
        # projection matmuls unblock after a handful of transfers ---
        wqkv_sb = [
            wop.tile([128, (G + 2) * HEAD_DIM], BF16, name=f"wqkv{i}")
            for i in range(NCT)
        ]
        # x split into tb0-slice + rest tiles (separate tiles, deps are
        # tile-granular) so tb0's projection pass only waits on 4.7MB
        # (weights + tb0 x slices), not the full 11MB
        xt0 = [xp.tile([128, TB], BF16, name=f"xt0_{i}") for i in range(NCT)]
        xtr = [xp.tile([128, T - TB], BF16, name=f"xtr{i}") for i in range(NCT)]
        for i in range(NCT):
            sl = slice(i * 128, (i + 1) * 128)
            nc.sync.dma_start(out=wqkv_sb[i], in_=wqkv_d.ap()[sl, :])
            nc.sync.dma_start(out=xt0[i], in_=xT_d.ap()[sl, :TB])
        for i in range(NCT):
            sl = slice(i * 128, (i + 1) * 128)
            nc.sync.dma_start(out=xtr[i], in_=xT_d.ap()[sl, TB:])

        ctab = const.tile([HEAD_DIM, T], BF16, name="ctab")
        stab = const.tile([HEAD_DIM, T], BF16, name="stab")
        nc.sync.dma_start(out=ctab, in_=ct_d.ap())
        nc.sync.dma_start(out=stab, in_=st_d.ap())
        ones_sb = const.tile([128, 128], BF16, name="ones_sb")
        ident = const.tile([128, 128], BF16, name="ident")
        trineg = const.tile([ST, ST], BF16, name="trineg")
        nc.sync.dma_start(out=ones_sb, in_=on_d.ap())
        nc.sync.dma_start(out=ident, in_=id_d.ap())
        nc.sync.dma_start(out=trineg, in_=tr_d.ap())
        if n_masks:
            msk_sb = const.tile([ST, n_masks * TB], BF16, name="msk_sb")
            for i in range(n_masks):
                nc.sync.dma_start(
                    out=msk_sb[:, i * TB : (i + 1) * TB],
                    in_=mk_d.ap()[i * ST : (i + 1) * ST, :],
                )
        wo_sb = [wop.tile([128, C], BF16, name=f"wo{h}") for h in range(G)]
        for h in range(G):
            nc.sync.dma_start(out=wo_sb[h], in_=wo_d.ap()[h * 128 : (h + 1) * 128, :])

        # per-t-block tiles (not one [128, T] tile) so stage-2 readers only
        # depend on the t-blocks they actually use — tile-granular dependency
        # tracking would otherwise serialize stage 2 behind ALL RoPE work
        qT = [
            [qkv.tile([128, TB], BF16, name=f"qT{h}_{tb}") for tb in range(NTB)]
            for h in range(G)
        ]
        kT = [qkv.tile([128, TB], BF16, name=f"kT{tb}") for tb in range(NTB)]
        vT = [qkv.tile([128, TB], BF16, name=f"vT{tb}") for tb in range(NTB)]
        vch = [qkv.tile([128, 128], BF16, name=f"v{s}") for s in range(NST)]

        # single PSUM pool: 8 x [128, 512] f32 bank tags, reused across stages
        ps = tc.alloc_tile_pool(name="ps", bufs=1, space="PSUM")

        def bank(tag):
            return ps.tile([128, TB], F32, name=tag, tag=tag)

        rpool = tc.alloc_tile_pool(name="rpool", bufs=3)

        # ---- stage 1: projections + RoPE + v transpose, pipelined per tb ----
        def emit_v_post(tb, v_ps):
            nc.vector.tensor_copy(vT[tb], v_ps)
            for r in range(4):
                s = 4 * tb + r
                vtp = bank(f"bk{6 + (r % 2)}").bitcast(BF16)[:, :128]
                nc.tensor.transpose(vtp, vT[tb][:, r * 128 : (r + 1) * 128], ident)
                nc.vector.tensor_copy(vch[s], vtp)

        def emit_rope(tb, dst, src_ps):
            tsl = slice(tb * TB, (tb + 1) * TB)
            nc.vector.tensor_copy(dst, src_ps)
            swp = rpool.tile([128, TB], BF16, name="swp", tag="swp")
            tmp = rpool.tile([128, TB], BF16, name="tmp", tag="tmp")
            nc.vector.stream_shuffle(swp, dst, SWAP_MASK)
            nc.vector.tensor_mul(tmp, dst, ctab[:, tsl])
            nc.vector.tensor_mul(swp, swp, stab[:, tsl])
            nc.vector.tensor_add(dst, tmp, swp)

        for tb in range(NTB):
            q_ps = [bank(f"bk{h}") for h in range(G)]
            k_ps = bank("bk4")
            v_ps = bank("bk5")

            def rhs_for(ci):
                return (
                    xt0[ci] if tb == 0 else xtr[ci][:, (tb - 1) * TB : tb * TB]
                )

            if tb == 0:
                # ci-major: tb0 is paced by the input DMA, so touch each
                # freshly-arrived chunk with all 6 matmuls at once
                for ci in range(NCT):
                    first, last = ci == 0, ci == NCT - 1
                    rhs = rhs_for(ci)
                    w = wqkv_sb[ci]
                    nc.tensor.matmul(
                        v_ps, lhsT=w[:, 640:768], rhs=rhs, start=first, stop=last
                    )
                    nc.tensor.matmul(
                        k_ps, lhsT=w[:, 512:640], rhs=rhs, start=first, stop=last
                    )
                    for h in range(G):
                        nc.tensor.matmul(
                            q_ps[h],
                            lhsT=w[:, h * 128 : (h + 1) * 128],
                            rhs=rhs,
                            start=first,
                            stop=last,
                        )
                emit_v_post(tb, v_ps)
                emit_rope(tb, kT[tb], k_ps)
                for h in range(G):
                    emit_rope(tb, qT[h][tb], q_ps[h])
            else:
                # per-tensor passes with the DVE consumer emitted right after
                # each pass: RoPE/transposes overlap THIS t-block's remaining
                # projections instead of piling up at the t-block boundary
                passes = [
                    (v_ps, slice(640, 768), lambda: emit_v_post(tb, v_ps)),
                    (k_ps, slice(512, 640), lambda: emit_rope(tb, kT[tb], k_ps)),
                ] + [
                    (
                        q_ps[h],
                        slice(h * 128, (h + 1) * 128),
                        (lambda h=h: emit_rope(tb, qT[h][tb], q_ps[h])),
                    )
                    for h in range(G)
                ]
                for ps_bank, wsl, post in passes:
                    for ci in range(NCT):
                        nc.tensor.matmul(
                            ps_bank,
                            lhsT=wqkv_sb[ci][:, wsl],
                            rhs=rhs_for(ci),
                            start=ci == 0,
                            stop=ci == NCT - 1,
                        )
                    post()

        # ---- stages 2+3: one global software pipeline over (tb, head, s-tile)
        # so head/t-block boundaries never drain the PE. Scores run LA s-tiles
        # ahead of the dependent den/PV matmuls (exp latency hidden). ----
        p2sb = tc.alloc_tile_pool(name="p2sb", bufs=6)  # ep tiles (LA+3 live)
        accp = tc.alloc_tile_pool(name="accp", bufs=2)  # DVE den accumulators
        phd = tc.alloc_tile_pool(name="phd", bufs=2)  # per-head den/oT/rcp
        p3sb = tc.alloc_tile_pool(name="p3sb", bufs=3)
        outp = tc.alloc_tile_pool(name="outp", bufs=10)
        state = {"score": 0, "head": 0, "ncopy": 0}
        oT_live: dict = {}

        def emit_oproj(tb):
            while chainq:  # all four heads' tiles must exist
                chainq.pop(0)[1]()
            oT_sbs = oT_live.pop(tb)
            for tch in range(TB // 128):
                # stage the full [128, C] row block in SBUF and ship it as ONE
                # DMA (4KB contiguous per partition -> fat packets; the tail
                # after the last matmul drains ~4x faster)
                osb = p3sb.tile([128, C], BF16, name="osb", tag="osb")
                for cb in range(C // 512):
                    ops = bank(f"bk{2 + (cb % 2)}")
                    for h in range(G):
                        nc.tensor.matmul(
                            ops,
                            lhsT=oT_sbs[h][:, tch * 128 : (tch + 1) * 128],
                            rhs=wo_sb[h][:, cb * 512 : (cb + 1) * 512],
                            start=h == 0,
                            stop=h == G - 1,
                        )
                    dst = osb[:, cb * 512 : (cb + 1) * 512]
                    # alternate the PSUM->SBUF copies between ScalarE and DVE
                    # so neither queue builds a backlog
                    if state["ncopy"] % 2 == 0:
                        nc.scalar.copy(dst, ops)
                    else:
                        nc.vector.tensor_copy(dst, ops)
                    state["ncopy"] += 1
                t0 = tb * TB + tch * 128
                # 4 partition-range DMAs: parallel queues AND 4KB packets
                for q in range(4):
                    nc.sync.dma_start(
                        out=out_d.ap()[t0 + q * 32 : t0 + (q + 1) * 32, :],
                        in_=osb[q * 32 : (q + 1) * 32, :],
                    )

        items = []  # (tb, h, idx)
        for tb in range(NTB):
            for h in range(G):
                for idx in range(len(plan[tb])):
                    items.append((tb, h, idx))

        ctx: dict = {}  # (tb,h) -> dict with oT_ps, den, eps

        def emit_score(it):
            tb, h, idx = it
            entries = plan[tb]
            s, w0, kind, mid = entries[idx]
            # bank roles chosen so stage-2 tiles reuse the PSUM banks that
            # stage-1's trailing (tb=3) DVE stream releases earliest:
            # v (bk5) and vtp (bk6/7) first -> stp; k (bk4) -> den;
            # q0/q1 (bk0/1) -> oT; q2/q3 (bk2/3) -> o_proj accumulators
            if idx == 0:
                ctx[(tb, h)] = {
                    "oT": bank(f"bk{0 + (state['head'] % 2)}"),
                    "den": bank("bk4"),
                    "eps": {},
                }
                if tb < 2:
                    # light t-blocks: denominator accumulates on DVE (lane
                    # sums) instead of per-tile ones-matmuls on the PE
                    ctx[(tb, h)]["acc"] = accp.tile(
                        [ST, TB], BF16, name="acc", tag=f"acc{state['head'] % 2}"
                    )
                state["head"] += 1
            stp = bank(f"bk{5 + (state['score'] % 3)}")
            state["score"] += 1
            diag = kind == DIAG
            nc.tensor.matmul(
                stp[:, w0:],
                lhsT=kT[s // 4][:, (s % 4) * 128 : (s % 4 + 1) * 128],
                rhs=qT[h][tb][:, w0:],
                start=True,
                stop=not diag,
                skip_group_check=diag,
            )
            if diag:
                # additive -512*(s>t) triangular mask folded into the score
                # accumulation on the PE (keeps DVE off the critical path);
                # exp then underflows to ~e-18 which is negligible in den/PV
                nc.tensor.matmul(
                    stp[:, w0 : w0 + ST],
                    lhsT=ident,
                    rhs=trineg,
                    start=False,
                    stop=True,
                    skip_group_check=True,
                )
            ep = p2sb.tile([ST, TB], BF16, name="ep", tag="ep")
            nc.scalar.activation(ep[:, w0:], stp[:, w0:], Exp, scale=SCALE)
            if kind == GEN:
                nc.vector.tensor_mul(
                    ep[:, w0:],
                    ep[:, w0:],
                    msk_sb[:, mid * TB : mid * TB + TB - w0],
                )
            ctx[(tb, h)]["eps"][idx] = ep

        chainq = []  # deferred end-of-head chains: [countdown, closure]

        def tick_chains():
            while chainq and chainq[0][0] <= 0:
                chainq.pop(0)[1]()
            for e in chainq:
                e[0] -= 1

        def emit_acc(it):
            tb, h, idx = it
            entries = plan[tb]
            s, w0, kind, mid = entries[idx]
            c = ctx[(tb, h)]
            ep = c["eps"].pop(idx)
            first, last = idx == 0, idx == len(entries) - 1
            if tb < 2:
                if first:
                    nc.vector.tensor_copy(c["acc"], ep)
                else:
                    nc.vector.tensor_add(
                        c["acc"][:, w0:], c["acc"][:, w0:], ep[:, w0:]
                    )
            else:
                nc.tensor.matmul(
                    c["den"][:, w0:],
                    lhsT=ones_sb,
                    rhs=ep[:, w0:],
                    start=first,
                    stop=last,
                    skip_group_check=True,
                )
            nc.tensor.matmul(
                c["oT"][:, w0:],
                lhsT=vch[s],
                rhs=ep[:, w0:],
                start=first,
                stop=last,
                skip_group_check=True,
            )
            tick_chains()
            if last:
                def chain(tb=tb, h=h, c=c):
                    if tb < 2:
                        # single cross-partition reduce of the DVE-built
                        # accumulator replaces the per-tile den matmuls
                        nc.tensor.matmul(
                            c["den"], lhsT=ones_sb, rhs=c["acc"],
                            start=True, stop=True,
                        )
                    # free both PSUM banks via ScalarE copies (short queue) so
                    # the PE's WAR on them never waits behind the DVE backlog
                    den_sb = phd.tile([128, TB], F32, name="den_sb", tag="den_sb")
                    nc.scalar.copy(den_sb, c["den"])
                    oT_f = phd.tile([128, TB], F32, name="oT_f", tag="oT_f")
                    nc.scalar.copy(oT_f, c["oT"])
                    rcp = phd.tile([128, TB], F32, name="rcp", tag="rcp")
                    # ~51-ULP approx is ample for the softmax denominator
                    nc.vector.reciprocal_approx_fast(rcp, den_sb)
                    oT_sb = outp.tile([128, TB], BF16, name="oT", tag="oT")
                    nc.vector.tensor_mul(oT_sb, oT_f, rcp)
                    oT_live.setdefault(tb, {})[h] = oT_sb
                    del ctx[(tb, h)]

                # tb1-h3 must run inline: its deferred den-final would
                # otherwise land inside tb2-h0's open bk4 den group
                if tb < 2 and not (tb == 1 and h == G - 1):
                    # defer 2 acc pops: the reduce's rhs (last DVE add) and
                    # the PE's WAR on the oT bank get ~1us of extra slack
                    chainq.append([2, chain])
                else:
                    chain()
                # o_proj for t-block tb is emitted two heads LATER (during
                # (tb+1, h1)'s attention) so its lhsT never waits on the
                # rescale chain of tb's last head
                if h == 1 and tb > 0:
                    emit_oproj(tb - 1)
                if tb == NTB - 1 and h == G - 1:
                    emit_oproj(tb)

        from collections import deque

        pend = deque()
        for it in items:
            emit_score(it)
            pend.append(it)
            if len(pend) > LA:
                emit_acc(pend.popleft())
        while pend:
            emit_acc(pend.popleft())
        while chainq:
            chainq.pop(0)[1]()

        outp.release()
        p3sb.release()
        phd.release()
        accp.release()
        p2sb.release()
        rpool.release()
        ps.release()
        xp.release()
        qkv.release()
        wop.release()
        const.release()

    nc.compile()
    return nc


def _prep_inputs(x, cos, sin, Wq, Wk, Wv, Wo, mask_tiles, n_masks):
    cos = np.asarray(cos, dtype=np.float32).reshape(T, HEAD_DIM // 2)
    sin = np.asarray(sin, dtype=np.float32).reshape(T, HEAD_DIM // 2)
    ctab = np.ascontiguousarray(np.repeat(cos, 2, axis=1).T).astype(BF)  # [128, T]
    s2 = np.repeat(sin, 2, axis=1)
    s2[:, 0::2] *= -1.0
    stab = np.ascontiguousarray(s2.T).astype(BF)
    trineg = (-512.0 * (np.arange(ST)[:, None] > np.arange(ST)[None, :])).astype(BF)

    xTb = [
        np.ascontiguousarray(np.asarray(x[b], dtype=np.float32).T).astype(BF)
        for b in range(B)
    ]
    in_maps = []
    for core in range(8):
        b, g = divmod(core, NUM_KV_HEADS)
        wqkv = np.concatenate(
            [
                Wq[:, g * 512 : (g + 1) * 512],
                Wk[:, g * 128 : (g + 1) * 128],
                Wv[:, g * 128 : (g + 1) * 128],
            ],
            axis=1,
        )
        m = {
            "xT": xTb[b],
            "wqkv": np.ascontiguousarray(wqkv).astype(BF),
            "wo": np.ascontiguousarray(Wo[g * 512 : (g + 1) * 512, :]).astype(BF),
            "ctab": ctab,
            "stab": stab,
            "ones": np.ones((128, 128), dtype=BF),
            "ident": np.eye(128, dtype=BF),
            "tri": trineg,
        }
        if n_masks:
            m["masks"] = mask_tiles.reshape(n_masks * ST, TB).astype(BF)
        in_maps.append(m)
    return in_maps


def kernel(x, cos, sin, mask, Wq, Wk, Wv, Wo, _trace=False, _result_box=None):
    from concourse.bass_utils import run_bass_kernel_spmd

    mask2d = np.asarray(mask).reshape(T, T).astype(bool)
    plan, mask_tiles = _classify_mask(mask2d)
    n_masks = int(mask_tiles.shape[0])

    key = (plan, n_masks)
    nc = _nc_cache.get(key)
    if nc is None:
        nc = _build(plan, n_masks)
        _nc_cache[key] = nc

    in_maps = _prep_inputs(x, cos, sin, Wq, Wk, Wv, Wo, mask_tiles, n_masks)
    res = run_bass_kernel_spmd(nc, in_maps, core_ids=list(range(8)), trace=_trace)
    if _result_box is not None:
        _result_box.append(res)

    out = np.zeros((B, T, C), dtype=np.float32)
    for core in range(8):
        b = core // NUM_KV_HEADS
        out[b] += res.results[core]["out"].astype(np.float32)
    return out

